# revision 38
# baseline (speedup 1.0000x reference)
"""Trainium2 Bass kernel for nn_AnyNetRefinement (disparity refinement with SPN scan).

Data-parallel over batch: core b processes image b end-to-end (no collectives).
Pipeline per core:
  conv1..conv3 (3x3+BN+ReLU, bf16, row-stacked PE matmuls). All DRAM
    intermediates are channel-interleaved [y, c, x] and matmul M-layouts are
    (yo-major, c-minor) so every load/store is one (row,channel)-merged
    big-first-dim DMA; loads on SP, epilogues alternate Act/DVE, stores on Pool.
  conv4 -> raw gates G; convd (disp -> 8ch feature, f32)
  normalize gates (|G1|+|G2|+|G3|) writing A taps + w0 directly into
    scan-resident SBUF tiles (work spread across Act/DVE/Pool)
  SPN left-to-right scan over W=640 on VectorE (folded [128=(c,hblock), 26] state,
    3-tap multiply into a slot buffer + 4-slot tensor_reduce (slot 3 = w0,
    pre-staged by ScalarE) + stream_shuffle halos)
  pf (incl. halo rows) exported block-mirrored to DRAM [128,26,WP]; convc runs
    on 24-row block groups straight from that layout (+disp, relu) -> out
"""

import numpy as np
import ml_dtypes

BF = ml_dtypes.bfloat16

H, W = 384, 640
HP, WP = 387, 642        # padded activation planes (+1 top/left, +2 bottom, +1 right)
X0S = (0, 320)
NX = 320                 # matmul free size (psum-bank safe)

_CACHE = {}
DO_MEMSET = True


# ---------------------------------------------------------------- host helpers
def _fold_bn(wt, g, b, m, v):
    s = g / np.sqrt(v + 1e-5)
    return (wt * s.reshape(-1, 1, 1, 1)).astype(np.float32), (b - m * s).astype(np.float32)


def _lhsT(wt, r_out, r_in, cin_g, npass):
    """lhsT [K=(cin_g,yi), npass, 3, M=(yo,cout)] (yo-major M for (y,c) stores)."""
    cout, cin = wt.shape[0], wt.shape[1]
    K = cin_g * r_in
    M = cout * r_out
    out = np.zeros((K, npass, 3, M), np.float32)
    for p in range(npass):
        for cg in range(cin_g):
            c = p * cin_g + cg
            if c >= cin:
                continue
            for dx in range(3):
                for yi in range(r_in):
                    k = cg * r_in + yi
                    for co in range(cout):
                        for yo in range(r_out):
                            dy = yi - yo
                            if 0 <= dy <= 2:
                                out[k, p, dx, yo * cout + co] = wt[co, c, dy, dx]
    return out.reshape(K, npass * 3 * M)


def _lhsT_blk(wt):
    """convc lhsT for 24-row block groups: K=(cg 0..3, yi 0..25), npass=2, M=24 rows.

    yi encodes block-local input row yi-1 (yi=0 is the up-halo row)."""
    out = np.zeros((4 * 26, 2, 3, 24), np.float32)
    for p in range(2):
        for cg in range(4):
            c = p * 4 + cg
            for dx in range(3):
                for yi in range(26):
                    for m in range(24):
                        dy = yi - m
                        if 0 <= dy <= 2:
                            out[cg * 26 + yi, p, dx, m] = wt[0, c, dy, dx]
    return out.reshape(104, 2 * 3 * 24)


def _pad_img(x, hp=HP, wp=WP):
    out = np.zeros((x.shape[0], hp, wp), BF)
    out[:, 1:1 + H, 1:1 + W] = x.astype(BF)
    return out


# ---------------------------------------------------------------- bass builder
def _build():
    import concourse.bass as bass
    import concourse.mybir as mybir
    from concourse import tile
    from concourse.vector_clock import ScopedClock

    f32 = mybir.dt.float32
    bf16 = mybir.dt.bfloat16
    ALU = mybir.AluOpType
    ACTF = mybir.ActivationFunctionType
    AX = mybir.AxisListType

    class TC(tile.TileContext):
        # this walrus build accepts only one sync-wait per Drain; split the
        # end-of-kernel waits across several drains.
        def _drain_and_barrier(self, tick_clock, wait_clock):
            nc = self.nc
            drain_inst = nc.sync.drain()
            wait_clock.add_sem_waits(drain_inst.ins, ScopedClock({None: tick_clock.global_clock}))
            waits = list(drain_inst.ins.sync_info.on_wait)
            if len(waits) > 1:
                drain_inst.ins.sync_info.on_wait = waits[:1]
                for i in range(1, len(waits)):
                    d2 = nc.sync.drain()
                    if d2.ins.sync_info is None:
                        d2.ins.sync_info = mybir.SyncInfo(on_wait=[waits[i]], on_update=[])
                    else:
                        d2.ins.sync_info.on_wait = [waits[i]]
            nc.all_engine_barrier()
            popped = nc._tile_sem_poison_stack.pop()
            assert popped is self._sem_poison
            nc.clear_and_free_semaphores(list(self.sems.allocated().values()))
            nc.all_engine_barrier()

    def dap(t, offset, dims):
        base = t if isinstance(t, bass.AP) else t[:]
        return bass.AP(base.tensor, base.offset + offset, [list(d) for d in dims])

    def sap(tile_ap, nparts, offset, dims, pstride=1):
        pstep = tile_ap.ap[0][0]
        return bass.AP(tile_ap.tensor, tile_ap.offset + offset,
                       [[pstep * pstride, nparts]] + [list(d) for d in dims])

    nc = bass.Bass("TRN2", num_swdge_queues=4)

    img = nc.declare_dram_parameter("img", [HP, 3, WP], bf16, isOutput=False)
    dpad = nc.declare_dram_parameter("dpad", [1, HP, WP], bf16, isOutput=False)
    dispf = nc.declare_dram_parameter("dispf", [H, W], f32, isOutput=False)
    w1k = nc.declare_dram_parameter("w1k", [30, 3 * 128], bf16, isOutput=False)
    w2k = nc.declare_dram_parameter("w2k", [128, 3 * 96], bf16, isOutput=False)
    w3k = nc.declare_dram_parameter("w3k", [128, 3 * 96], bf16, isOutput=False)
    w4k = nc.declare_dram_parameter("w4k", [112, 3 * 120], bf16, isOutput=False)
    wdk = nc.declare_dram_parameter("wdk", [18, 3 * 128], bf16, isOutput=False)
    wck = nc.declare_dram_parameter("wck", [104, 2 * 3 * 24], bf16, isOutput=False)
    b1v = nc.declare_dram_parameter("b1v", [128, 1], f32, isOutput=False)
    b2v = nc.declare_dram_parameter("b2v", [96, 1], f32, isOutput=False)
    b3v = nc.declare_dram_parameter("b3v", [96, 1], f32, isOutput=False)
    outp = nc.declare_dram_parameter("out", [H, W], f32, isOutput=True)

    with TC(nc) as tc:
        with (tc.tile_pool(name="dram", bufs=1, space="DRAM") as dram,
              tc.tile_pool(name="wts", bufs=1) as pw):
            act1 = dram.tile([HP, 16, WP], bf16, tag="act1")
            act2 = dram.tile([HP, 16, WP], bf16, tag="act2")
            act3 = dram.tile([HP, 16, WP], bf16, tag="act3")
            Gt = dram.tile([385, 24, W], bf16, tag="G")
            dfeat = dram.tile([H, 8, W], f32, tag="dfeat")
            ppadB = dram.tile([128, 26, WP], bf16, tag="ppadB")

            # ---------------- weights/biases
            wtl = {}
            for nm, prm, kk, nm3 in (("w1", w1k, 30, 3 * 128), ("w2", w2k, 128, 3 * 96),
                                     ("w3", w3k, 128, 3 * 96), ("w4", w4k, 112, 3 * 120),
                                     ("wd", wdk, 18, 3 * 128), ("wc", wck, 104, 6 * 24)):
                t = pw.tile([kk, nm3], bf16, tag=f"{nm}t", name=f"{nm}t")
                nc.sync.dma_start(out=t[:], in_=prm[:])
                wtl[nm] = t
            b1t = pw.tile([128, 1], f32, tag="b1t")
            nc.sync.dma_start(out=b1t[:], in_=b1v[:])
            b2t = pw.tile([96, 1], f32, tag="b2t")
            nc.sync.dma_start(out=b2t[:], in_=b2v[:])
            b3t = pw.tile([96, 1], f32, tag="b3t")
            nc.sync.dma_start(out=b3t[:], in_=b3v[:])

            # ---------------- zero row-borders of padded internal buffers
            zt = pw.tile([128, 2 * WP], bf16, tag="zt")
            nc.vector.memset(zt[:], 0.0)
            ztv = zt[:]
            for buf, cc in ((act1, 16), (act2, 16), (act3, 16)):
                # top row 0 (all c) + bottom rows 385,386: contiguous (y,c,x) runs
                nc.sync.dma_start(out=dap(buf, 0, [(WP, cc), (1, WP)]),
                                  in_=sap(ztv, cc, 0, [(1, WP)]))
                nc.sync.dma_start(out=dap(buf, 385 * cc * WP, [(WP, 2 * cc), (1, WP)]),
                                  in_=sap(ztv, 2 * cc, 0, [(1, WP)]))
                # col stripes x=0 / x=WP-1 over all (y,c)
                for col in (0, WP - 1):
                    nc.gpsimd.dma_start(out=dap(buf, col, [(WP, HP * cc), (1, 1)]),
                                        in_=sap(ztv, 1, 0, [(0, HP * cc), (1, 1)]))

            # ---------------- generic conv
            rhs_eng = [0]
            st_eng = [0]

            def conv(prhs, pout, ppsum, src, dst, wt, wK, wM, btile, cin_g, npass,
                     cout, r, rin, S, GS,
                     relu, src_c, dst_w, dst_pad, out_f32=False):
                K = cin_g * rin
                assert K == wK
                M = cout * r
                assert M == wM
                wv = wt[:]
                g0 = 0
                while g0 < S:
                    nsl = min(GS, S - g0)
                    y0 = r * g0
                    rhss = []
                    for p_ in range(npass):
                        rt = prhs.tile([K, GS, WP], bf16, tag="rhs", name="rhs")
                        for sl in range(nsl):
                            eng = nc.sync
                            rhs_eng[0] += 1
                            eng.dma_start(
                                out=rt[:, sl, :],
                                in_=dap(src, (y0 + sl * r) * src_c * WP + p_ * cin_g * WP,
                                        [(WP, cin_g), (src_c * WP, rin), (1, WP)]))
                        rhss.append(rt)
                    ps = []
                    for sl in range(nsl):
                        pstile = ppsum.tile([128, 2 * NX], f32, tag="ps", name="ps")
                        ps.append(pstile)
                    for x0, nxw in ((0, 512), (512, 128)):
                        for p_ in range(npass):
                            for dx in range(3):
                                for sl in range(nsl):
                                    nc.tensor.matmul(
                                        ps[sl][:M, x0:x0 + nxw],
                                        sap(wv, K, (p_ * 3 + dx) * M, [(1, M)]),
                                        rhss[p_][:, sl, x0 + dx:x0 + dx + nxw],
                                        start=(p_ == 0 and dx == 0),
                                        stop=(p_ == npass - 1 and dx == 2))
                    ot = pout.tile([M, GS, 2 * NX], f32 if out_f32 else bf16, tag="cout", name="cout")
                    for sl in range(nsl):
                        p = ps[sl][:M, :]
                        o = ot[:, sl, :]
                        st_eng[0] += 1
                        if relu:
                            if st_eng[0] % 2 == 0:
                                nc.scalar.activation(o, p, ACTF.Relu, bias=btile[:M, :], scale=1.0)
                            else:
                                nc.vector.tensor_scalar(o, p, btile[:M, :], 0.0, ALU.add, ALU.max)
                        else:
                            if st_eng[0] % 2 == 0:
                                nc.scalar.copy(o, p)
                            else:
                                nc.vector.tensor_copy(o, p)
                    for sl in range(nsl):
                        pad = 1 if dst_pad else 0
                        nc.gpsimd.dma_start(
                            out=dap(dst, (pad + y0 + sl * r) * cout * dst_w + pad,
                                    [(dst_w, cout * r), (1, 2 * NX)]),
                            in_=ot[:, sl, :])
                    g0 += nsl

            with (tc.tile_pool(name="rhs", bufs=16) as prhs,
                  tc.tile_pool(name="cout", bufs=16) as pout,
                  tc.tile_pool(name="psum", bufs=4, space="PSUM") as ppsum):
                P3 = (prhs, pout, ppsum)
                conv(*P3, img, act1, wtl["w1"], 30, 128, b1t, 3, 1, 16, 8, 10, 48, 1,
                     True, 3, WP, True)
                conv(*P3, act1, act2, wtl["w2"], 128, 96, b2t, 16, 1, 16, 6, 8, 64, 1,
                     True, 16, WP, True)
                conv(*P3, act2, act3, wtl["w3"], 128, 96, b3t, 16, 1, 16, 6, 8, 64, 1,
                     True, 16, WP, True)
                conv(*P3, act3, Gt, wtl["w4"], 112, 120, None, 16, 1, 24, 5, 7, 77, 1,
                     False, 16, W, False)
                conv(*P3, dpad, dfeat, wtl["wd"], 18, 128, None, 1, 1, 8, 16, 18, 24, 1,
                     False, 1, W, False, out_f32=True)

            # ---------------- scan-resident gate/w0 tiles
            with tc.tile_pool(name="scanbig", bufs=1) as pbig:
                gw = pbig.tile([128, 3, 24, W], bf16, tag="gw")
                gwv = gw[:]
                w0w = pbig.tile([128, 24, W], bf16, tag="w0w")
                w0v = w0w[:]

                # ---------------- gate normalization (direct into gw/w0w)
                NCH = 12
                CH = 15360 // NCH   # = 1280 = 2 rows x 640
                with (tc.tile_pool(name="norm3", bufs=6) as pn3,
                      tc.tile_pool(name="norm1", bufs=2) as pn1):
                    for k in range(NCH):
                        gp = []
                        for tap in range(3):
                            g2 = pn3.tile([128, CH], bf16, tag="gld", name="gld")
                            for rr in range(2):
                                leng = nc.sync
                                leng.dma_start(
                                    out=g2[:, rr * W:(rr + 1) * W],
                                    in_=dap(Gt, tap * 8 * W + (k * 2 + rr) * 24 * W,
                                            [(W, 8), (24 * 24 * W, 16), (1, W)]))
                            gp.append(g2)
                        gts = [gp[tap][:] for tap in range(3)]
                        ab = []
                        for tap in range(3):
                            a = pn3.tile([128, CH], bf16, tag="gabs", name="gabs")
                            nc.scalar.activation(a[:], gts[tap][:], ACTF.Abs)
                            ab.append(a)
                        s12 = pn1.tile([128, CH], bf16, tag="s12")
                        nc.gpsimd.tensor_tensor(out=s12[:], in0=ab[0][:], in1=ab[1][:], op=ALU.add)
                        sf = pn1.tile([128, CH], f32, tag="sf")
                        nc.vector.scalar_tensor_tensor(out=sf[:], in0=ab[2][:], scalar=1e-8,
                                                       in1=s12[:], op0=ALU.add, op1=ALU.add)
                        rs = pn1.tile([128, CH], f32, tag="rs")
                        nc.vector.reciprocal(rs[:], sf[:])
                        gsl = []
                        for tap in range(3):
                            o = sap(gwv, 128, tap * 24 * W + k * CH, [(1, CH)])
                            teng = nc.gpsimd if tap == 0 else nc.vector
                            teng.tensor_tensor(out=o, in0=gts[tap][:], in1=rs[:], op=ALU.mult)
                            gsl.append(o)
                        a12 = pn1.tile([128, CH], bf16, tag="a12")
                        nc.gpsimd.tensor_tensor(out=a12[:], in0=gsl[0], in1=gsl[1], op=ALU.add)
                        asum = pn1.tile([128, CH], bf16, tag="asum")
                        nc.gpsimd.tensor_tensor(out=asum[:], in0=a12[:], in1=gsl[2], op=ALU.add)
                        t2 = pn1.tile([128, CH], bf16, tag="t2")
                        nc.scalar.activation(t2[:], asum[:], ACTF.Copy, bias=1.0, scale=-1.0)
                        df = pn1.tile([128, CH], f32, tag="dfl")
                        for rr in range(2):
                            leng = nc.sync
                            leng.dma_start(
                                out=df[:, rr * W:(rr + 1) * W],
                                in_=dap(dfeat, (k * 2 + rr) * 8 * W,
                                        [(W, 8), (24 * 8 * W, 16), (1, W)]))
                        nc.vector.tensor_tensor(out=sap(w0v, 128, k * CH, [(1, CH)]),
                                                in0=t2[:], in1=df[:], op=ALU.mult)
                    # zero edge gates: tap0 (up) at row 0 of hb=0; tap2 (dn) at row 23 of hb=15
                    pstep = gwv.ap[0][0]
                    for c_ in range(8):
                        nc.sync.dma_start(
                            out=bass.AP(gwv.tensor, gwv.offset + (16 * c_) * pstep,
                                        [[pstep, 1], [1, W]]),
                            in_=sap(ztv, 1, 0, [(1, W)]))
                        nc.sync.dma_start(
                            out=bass.AP(gwv.tensor,
                                        gwv.offset + (16 * c_ + 15) * pstep + 2 * 24 * W + 23 * W,
                                        [[pstep, 1], [1, W]]),
                            in_=sap(ztv, 1, 0, [(1, W)]))

                # ---------------- SPN scan
                mask_up = [(i - 1) % 32 for i in range(32)]
                mask_dn = [(i + 1) % 32 for i in range(32)]
                TB = 32  # w0-staging chunk
                with tc.tile_pool(name="scansm", bufs=1) as psm:
                    # pf: prop buffer doubling as scan state (bf16).
                    # rows: 0 = up-halo, 1..24 = block rows, 25 = dn-halo.
                    # col 1+t holds h_t; col 0 = zero initial state.
                    pf = psm.tile([128, 26, WP], bf16, tag="pf")
                    pfv = pf[:]
                    nc.vector.memset(sap(pfv, 128, 0, [(WP, 26), (WP - 1, 2)]), 0.0)
                    prw = psm.tile([128, 2, TB, 24, 4], f32, tag="prw")
                    prv = prw[:]
                    for t in range(W):
                        j = t % TB
                        bi = (t // TB) % 2
                        if j == 0:
                            nc.scalar.copy(
                                sap(prv, 128, bi * (TB * 96) + 3, [(4, 24), (96, TB)]),
                                sap(w0v, 128, t, [(W, 24), (1, TB)]))
                        base = bi * (TB * 96) + j * 96
                        taps = sap(pfv, 128, t, [(WP, 24), (WP, 3)])
                        g_t = sap(gwv, 128, t, [(W, 24), (24 * W, 3)])
                        nc.vector.tensor_tensor(out=sap(prv, 128, base, [(4, 24), (1, 3)]),
                                                in0=g_t, in1=taps, op=ALU.mult)
                        with nc.allow_low_precision(reason="bf16 scan state, validated"):
                            nc.vector.tensor_reduce(out=sap(pfv, 128, WP + 1 + t, [(WP, 24)]),
                                                    in_=sap(prv, 128, base, [(4, 24), (1, 4)]),
                                                    axis=AX.X, op=ALU.add)
                        nc.vector.stream_shuffle(out=sap(pfv, 128, 1 + t, [(1, 1)]),
                                                 in_=sap(pfv, 128, 24 * WP + 1 + t, [(1, 1)]),
                                                 mask=mask_up)
                        nc.vector.stream_shuffle(out=sap(pfv, 128, 25 * WP + 1 + t, [(1, 1)]),
                                                 in_=sap(pfv, 128, WP + 1 + t, [(1, 1)]),
                                                 mask=mask_dn)
                    # export full pf (incl. halo rows) block-mirrored: [128,26,WP]
                    nc.scalar.dma_start(
                        out=dap(ppadB, 0, [(26 * WP, 128), (1, 26 * WP)]),
                        in_=sap(pfv, 128, 0, [(1, 26 * WP)]))
                    # zero the two garbage (channel-wrapped) halo rows:
                    # up-halo of hb=0 blocks, dn-halo of hb=15 blocks
                    nc.sync.dma_start(
                        out=dap(ppadB, 0, [(16 * 26 * WP, 8), (1, WP)]),
                        in_=sap(ztv, 8, 0, [(1, WP)]))
                    nc.sync.dma_start(
                        out=dap(ppadB, (15 * 26 + 25) * WP, [(16 * 26 * WP, 8), (1, WP)]),
                        in_=sap(ztv, 8, 0, [(1, WP)]))

            # ---------------- convc: ppadB -> out (+disp, relu), 24-row blocks
            with (tc.tile_pool(name="rhsc", bufs=16) as prhs2,
                  tc.tile_pool(name="coutc", bufs=16) as pout2,
                  tc.tile_pool(name="psumc", bufs=4, space="PSUM") as ppsum2):
                wcv = wtl["wc"][:]
                for g in range(16):
                    rts = []
                    for p_ in range(2):
                        rt = prhs2.tile([104, WP], bf16, tag="rhsc", name="rhsc")
                        nc.scalar.dma_start(
                            out=rt[:],
                            in_=dap(ppadB, (p_ * 4 * 16 + g) * 26 * WP,
                                    [(16 * 26 * WP, 4), (WP, 26), (1, WP)]))
                        rts.append(rt)
                    pstile = ppsum2.tile([24, 2 * NX], f32, tag="psc", name="psc")
                    for x0, nxw in ((0, 512), (512, 128)):
                        for p_ in range(2):
                            for dx in range(3):
                                nc.tensor.matmul(
                                    pstile[:24, x0:x0 + nxw],
                                    sap(wcv, 104, (p_ * 3 + dx) * 24, [(1, 24)]),
                                    rts[p_][:, x0 + dx:x0 + dx + nxw],
                                    start=(p_ == 0 and dx == 0),
                                    stop=(p_ == 1 and dx == 2))
                    dt_ = pout2.tile([24, 2 * NX], f32, tag="dtile", name="dtile")
                    nc.scalar.dma_start(out=dt_[:],
                                        in_=dap(dispf, g * 24 * W, [(W, 24), (1, 2 * NX)]))
                    tmp = pout2.tile([24, 2 * NX], f32, tag="ctmp", name="ctmp")
                    nc.vector.tensor_tensor(out=tmp[:], in0=pstile[:24, :], in1=dt_[:], op=ALU.add)
                    ot = pout2.tile([24, 2 * NX], f32, tag="coutc", name="coutc")
                    nc.vector.tensor_scalar(ot[:], tmp[:], 0.0, None, ALU.max)
                    st_eng[0] += 1
                    seng = nc.gpsimd if st_eng[0] % 2 == 0 else nc.sync
                    seng.dma_start(out=dap(outp, g * 24 * W, [(W, 24), (1, 2 * NX)]),
                                   in_=ot[:])

    # Engine-sem update thinning: Tile increments each engine's sem on every
    # op, but only values that some wait references matter. Dropping the rest
    # (and renumbering waits to the kept-update count at the same producer op)
    # is semantically exact and shrinks the sem-update backlog.
    ENG_SEMS = ("DVE_44", "PE_44", "Activation_44", "Pool_44", "SP_44")
    insts_all = []
    for fn in nc.m.functions:
        for bb in fn.blocks:
            insts_all.extend(bb.instructions)
    waited = {sm: set() for sm in ENG_SEMS}
    for inst in insts_all:
        si = inst.sync_info
        if si and si.on_wait:
            for wt_ in si.on_wait:
                if wt_.ant_name in waited:
                    waited[wt_.ant_name].add(wt_.wait_value)
    cum = {sm: 0 for sm in ENG_SEMS}
    newcum = {sm: 0 for sm in ENG_SEMS}
    remap = {sm: {} for sm in ENG_SEMS}
    for inst in insts_all:
        si = inst.sync_info
        if not si:
            continue
        ups = si.on_update
        if ups:
            keep = []
            for u in ups:
                sm = u.ant_name
                if sm in cum:
                    cum[sm] += 1
                    if cum[sm] in waited[sm]:
                        newcum[sm] += 1
                        remap[sm][cum[sm]] = newcum[sm]
                        keep.append(u)
                    # else: drop this update
                else:
                    keep.append(u)
            if len(keep) != len(ups):
                si.on_update = keep
    for inst in insts_all:
        si = inst.sync_info
        if si and si.on_wait:
            ws = list(si.on_wait)
            ch = False
            for i_, wt_ in enumerate(ws):
                if wt_.ant_name in remap and wt_.wait_value in remap[wt_.ant_name]:
                    nv = remap[wt_.ant_name][wt_.wait_value]
                    if nv != wt_.wait_value:
                        wt_.wait_value = nv
                        ch = True
            if ch:
                si.on_wait = ws

    # walrus on this stack accepts at most one sync wait per instruction:
    # spill excess waits onto same-engine NOPs injected just before.
    nwn = [0]
    for fn in nc.m.functions:
        for bb in fn.blocks:
            il = bb.instructions
            i = 0
            while i < len(il):
                inst = il[i]
                si = inst.sync_info
                if si is not None and si.on_wait and len(si.on_wait) > 1:
                    waits = list(si.on_wait)
                    si.on_wait = waits[-1:]
                    for j in range(len(waits) - 1):
                        nwn[0] += 1
                        nop = mybir.InstNoOp(name=f"WS-{nwn[0]}", ins=[], outs=[])
                        nop.engine = inst.engine
                        nop.sync_info = mybir.SyncInfo(on_wait=[waits[j]], on_update=[])
                        nc.register_instruction(nop, overwrite=True)
                        il.insert(i, nop)
                        i += 1
                i += 1

    return nc


def _prep_inputs(inputs):
    w1, b1 = _fold_bn(inputs['w1'], inputs['bn1_g'], inputs['bn1_b'], inputs['bn1_m'], inputs['bn1_v'])
    w2, b2 = _fold_bn(inputs['w2'], inputs['bn2_g'], inputs['bn2_b'], inputs['bn2_m'], inputs['bn2_v'])
    w3, b3 = _fold_bn(inputs['w3'], inputs['bn3_g'], inputs['bn3_b'], inputs['bn3_m'], inputs['bn3_v'])

    w1k = _lhsT(w1, 8, 10, 3, 1).astype(BF)                       # [30, 384]
    w2k = _lhsT(w2, 6, 8, 16, 1).astype(BF)                       # [128, 288]
    w3k = _lhsT(w3, 6, 8, 16, 1).astype(BF)
    w4k = _lhsT(inputs['w4'].astype(np.float32), 5, 7, 16, 1).astype(BF)   # [112, 360]
    wdk = _lhsT(inputs['wd'].astype(np.float32), 16, 18, 1, 1).astype(BF)  # [18, 384]
    wck = _lhsT_blk(inputs['wc'].astype(np.float32)).astype(BF)            # [104, 144]

    b1r = np.tile(b1, 8).reshape(128, 1).astype(np.float32)
    b2r = np.tile(b2, 6).reshape(96, 1).astype(np.float32)
    b3r = np.tile(b3, 6).reshape(96, 1).astype(np.float32)

    maps = []
    for b in range(8):
        maps.append({
            "img": np.ascontiguousarray(np.transpose(_pad_img(inputs['leftImage'][b]), (1, 0, 2))),
            "dpad": _pad_img(inputs['disp'][b]),
            "dispf": inputs['disp'][b, 0].astype(np.float32),
            "w1k": w1k, "w2k": w2k, "w3k": w3k, "w4k": w4k, "wdk": wdk, "wck": wck,
            "b1v": b1r, "b2v": b2r, "b3v": b3r,
        })
    return maps


def kernel(**inputs):
    from concourse.bass_utils import run_bass_kernel_spmd

    if "nc" not in _CACHE:
        _CACHE["nc"] = _build()
    nc = _CACHE["nc"]
    maps = _prep_inputs(inputs)
    res = run_bass_kernel_spmd(nc, maps, core_ids=list(range(8)))
    out = np.stack([res.results[i]["out"] for i in range(8)])[:, None].astype(np.float32)
    return out

